# revision 15
# baseline (speedup 1.0000x reference)
"""GATv2 2-layer GNN on 8 Trainium2 NeuronCores (Bass/Tile, edge-parallel).

Sharding: edges sorted by dst node, dst-range sharded across 8 cores
(core k owns dst nodes [1250k, 1250(k+1))); self-loop edges (attr =
host-computed mean of incoming attrs) are merged into the stream.

Layer-1 key trick: aggregate-then-project. Since
  out[d] = sum_e alpha_e * (x[src_e] @ Wl1) = (sum_e alpha_e x[src_e]) @ Wl1,
the per-edge payload stays F=128 wide; the 128->2048 projection runs once
per 125-dst-node group. Per-edge logits q = x_src@Wl1 + x_dst@Wr1 + ea@We1
are assembled directly in PSUM from three streamed matmul sources
(src/dst x-rows are host-gathered per edge, per the edge-parallel
sharding hint), drained through a fused LeakyRelu on ScalarE, and
reduced per head with one multiply + one segmented reduce on DVE.
No node-projection tables, no indirect gathers, and no DMA transposes
in layer 1. Between layers one AllGather exchanges the 64-wide layer-2
projections.
"""
import sys
sys.path.insert(0, "/opt/trn_rl_repo")

import numpy as np
import ml_dtypes

import concourse.bass as bass
import concourse.bacc as bacc
import concourse.tile as tile
from concourse import mybir
from concourse.bass_utils import run_bass_kernel_spmd

BF16 = ml_dtypes.bfloat16

N, E, F = 10000, 80000, 128
H1, C1 = 8, 256
D1 = H1 * C1          # 2048
D2 = 32               # layer-2 out (1 head)
NEG = 0.2
M = 8                 # cores
NPC = N // M          # 1250 nodes per core
GN = 125              # dst nodes per group
G = NPC // GN         # 10 groups per core
P = 128

dt = mybir.dt


def _build_program(CH, phases="ABC"):
    """SPMD Bass program. CH = 128-edge chunks per 125-dst-node group."""
    L = G * CH * P
    nc = bacc.Bacc("TRN2", target_bir_lowering=False, debug=False, num_devices=M)

    ei = {}
    def EIN(name, shape, dtype):
        ei[name] = nc.dram_tensor(name, list(shape), dtype, kind="ExternalInput")
        return ei[name]

    xsrcT = EIN("xsrcT", (P, L), dt.bfloat16)   # x[src]^T  (f-major lhsT)
    xdstT = EIN("xdstT", (P, L), dt.bfloat16)   # x[dst]^T
    eaT   = EIN("eaT",   (P, L), dt.bfloat16)   # edge_attr^T (incl self attrs)
    xsrcN = EIN("xsrcN", (P, L), dt.bfloat16)   # x[src] chunk-blocked [e, f]
    s01N  = EIN("s01N",  (P, L), dt.bfloat16)   # onehot chunk-blocked [e, d]
    s01T  = EIN("s01T",  (P, L), dt.bfloat16)   # onehot d-major [d, e]
    srci  = EIN("srci",  (P, G * CH), dt.int32) # global src id, chunk-blocked
    wl1   = EIN("wl1",   (F, D1), dt.bfloat16)
    wr1   = EIN("wr1",   (F, D1), dt.bfloat16)
    we1   = EIN("we1",   (F, D1), dt.bfloat16)
    att1s = EIN("att1s", (P, D1), dt.bfloat16)  # att1 row-replicated
    wl2b  = EIN("wl2b",  (P, 16 * D2), dt.bfloat16)  # [p, k*32+c] = Wl2[k*128+p, c]
    wr2b  = EIN("wr2b",  (P, 16 * D2), dt.bfloat16)
    we2   = EIN("we2",   (F, D2), dt.bfloat16)
    att2s = EIN("att2s", (P, D2), dt.bfloat16)
    ident = EIN("ident", (P, P), dt.bfloat16)

    out = nc.dram_tensor("out", [NPC, D2], dt.float32, kind="ExternalOutput")
    xlr2 = nc.dram_tensor("xlr2", [NPC, 2 * D2], dt.float32)
    ag = nc.dram_tensor("ag", [N, 2 * D2], dt.float32, addr_space="Shared")
    xl2_tab = nc.dram_tensor("xl2_tab", [N, D2], dt.float32)

    AF = mybir.ActivationFunctionType
    ALU = mybir.AluOpType
    AX = mybir.AxisListType

    with tile.TileContext(nc) as tc:
        with tc.tile_pool(name="consts", bufs=1) as cp:
            def CONST(src, shape, tag, dtype=dt.bfloat16):
                t = cp.tile(list(shape), dtype, tag=tag)
                nc.sync.dma_start(out=t[:, :], in_=src[:, :])
                return t
            wl1_sb = CONST(wl1, (F, D1), "wl1")
            wr1_sb = CONST(wr1, (F, D1), "wr1")
            we1_sb = CONST(we1, (F, D1), "we1")
            att1_sb = CONST(att1s, (P, D1), "att1")
            wl2_sb = CONST(wl2b, (P, 16 * D2), "wl2")
            wr2_sb = CONST(wr2b, (P, 16 * D2), "wr2")
            we2_sb = CONST(we2, (F, D2), "we2")
            att2_sb = CONST(att2s, (P, D2), "att2")
            id_sb = CONST(ident, (P, P), "ident")
            xr2sb = cp.tile([P, G * D2], dt.bfloat16)  # layer-2 dst proj, per group

            with (
                tc.tile_pool(name="accp", bufs=1, space="PSUM") as accp,  # 2 banks
                tc.tile_pool(name="qp", bufs=2, space="PSUM") as qp,      # 2 banks
                tc.tile_pool(name="epi", bufs=1, space="PSUM") as epi,    # 4 banks
                tc.tile_pool(name="gin", bufs=2) as gin,
                tc.tile_pool(name="wk", bufs=2) as wk,
                tc.tile_pool(name="wk3", bufs=3) as wk3,
            ):
                # Pre-touch every (pool, tag, shape) so pool regions are fully
                # sized before any later pool allocates (avoids mid-stream pool
                # growth, which deadlocks the tile scheduler).
                PRETOUCH = (
                    (gin, "xsT", (P, CH * P), dt.bfloat16),
                    (gin, "xdT", (P, CH * P), dt.bfloat16),
                    (gin, "eaT", (P, CH * P), dt.bfloat16),
                    (gin, "xsN", (P, CH * P), dt.bfloat16),
                    (gin, "s01", (P, CH * P), dt.bfloat16),
                    (gin, "s1T", (P, CH * P), dt.bfloat16),
                    (wk, "exall", (P, CH * H1), dt.bfloat16),
                    (wk, "m", (P, D1), dt.bfloat16),
                    (wk, "w", (P, D1), dt.bfloat16),
                    (wk, "logit", (P, H1), dt.float32),
                    (wk, "ex", (P, H1), dt.float32),
                    (wk, "xh", (P, 8 * P), dt.bfloat16),
                    (wk, "dninv", (P, H1), dt.float32),
                    (wk, "xagg", (P, 8 * P), dt.bfloat16),
                    (wk, "xaTs", (P, 8 * P), dt.bfloat16),
                    (wk, "hTs", (P, 16 * P), dt.bfloat16),
                    (wk, "xlr2", (P, 2 * D2), dt.float32),
                    (wk, "si", (P, CH), dt.int32),
                    (wk, "d2i", (P, 1), dt.float32),
                    (wk, "o2", (P, D2), dt.float32),
                    (wk3, "xl2e", (P, D2), dt.float32),
                    (wk3, "q2b", (P, D2), dt.bfloat16),
                    (wk3, "m2", (P, D2), dt.bfloat16),
                    (wk3, "w2", (P, D2), dt.bfloat16),
                    (wk3, "lg2", (P, 1), dt.float32),
                    (wk3, "ex2", (P, 1), dt.float32),
                    (wk3, "xls2", (P, D2 + 1), dt.bfloat16),
                )
                for pool, tag, shape, dtype in PRETOUCH:
                    t = pool.tile(list(shape), dtype, tag=tag)
                    nc.vector.memset(t[:, :], 0)
                # ============ layer 1: edge pass, aggregate-then-project ======
                for g in range(G):
                    g0 = g * CH * P
                    xsT_g = gin.tile([P, CH * P], dt.bfloat16, tag="xsT")
                    nc.sync.dma_start(out=xsT_g[:, :], in_=xsrcT[:, g0:g0 + CH * P])
                    xdT_g = gin.tile([P, CH * P], dt.bfloat16, tag="xdT")
                    nc.sync.dma_start(out=xdT_g[:, :], in_=xdstT[:, g0:g0 + CH * P])
                    eaT_g = gin.tile([P, CH * P], dt.bfloat16, tag="eaT")
                    nc.sync.dma_start(out=eaT_g[:, :], in_=eaT[:, g0:g0 + CH * P])
                    xsN_g = gin.tile([P, CH * P], dt.bfloat16, tag="xsN")
                    nc.sync.dma_start(out=xsN_g[:, :], in_=xsrcN[:, g0:g0 + CH * P])
                    s01_g = gin.tile([P, CH * P], dt.bfloat16, tag="s01")
                    nc.sync.dma_start(out=s01_g[:, :], in_=s01N[:, g0:g0 + CH * P])

                    acc = accp.tile([P, 8 * P], dt.float32, tag="acc")
                    exall = wk.tile([P, CH * H1], dt.bfloat16, tag="exall")

                    for ch in range(CH):
                        sl = slice(ch * P, (ch + 1) * P)
                        m_t = wk.tile([P, D1], dt.bfloat16, tag="m")
                        for j in range(4):
                            q = qp.tile([P, 512], dt.float32, tag="q")
                            cs = slice(512 * j, 512 * (j + 1))
                            nc.tensor.matmul(out=q[:, :], lhsT=eaT_g[:, sl],
                                             rhs=we1_sb[:, cs], start=True, stop=False)
                            nc.tensor.matmul(out=q[:, :], lhsT=xsT_g[:, sl],
                                             rhs=wl1_sb[:, cs], start=False, stop=False)
                            nc.tensor.matmul(out=q[:, :], lhsT=xdT_g[:, sl],
                                             rhs=wr1_sb[:, cs], start=False, stop=True)
                            nc.scalar.activation(out=m_t[:, cs], in_=q[:, :],
                                                 func=AF.Prelu, alpha=NEG)
                        w_t = wk.tile([P, D1], dt.bfloat16, tag="w")
                        nc.vector.tensor_tensor(out=w_t[:, :], in0=m_t[:, :],
                                                in1=att1_sb[:, :], op=ALU.mult)
                        logit = wk.tile([P, H1], dt.float32, tag="logit")
                        nc.vector.tensor_reduce(
                            out=logit[:, :],
                            in_=w_t[:, :].rearrange("p (h c) -> p h c", h=H1),
                            axis=AX.X, op=ALU.add)
                        ex = wk.tile([P, H1], dt.float32, tag="ex")
                        nc.scalar.activation(out=ex[:, :], in_=logit[:, :], func=AF.Exp)
                        nc.vector.tensor_copy(out=exall[:, ch * H1:(ch + 1) * H1],
                                              in_=ex[:, :])
                        xh = wk.tile([P, 8 * P], dt.bfloat16, tag="xh")
                        for h in range(H1):
                            nc.gpsimd.tensor_scalar(
                                out=xh[:, h * P:(h + 1) * P], in0=xsN_g[:, sl],
                                scalar1=ex[:, h:h + 1], scalar2=None, op0=ALU.mult)
                        for half in range(2):
                            nc.tensor.matmul(
                                out=acc[:, half * 512:(half + 1) * 512],
                                lhsT=s01_g[:, sl],
                                rhs=xh[:, half * 512:(half + 1) * 512],
                                start=(ch == 0), stop=(ch == CH - 1))

                    # ---- group epilogue: softmax denom, normalize, project ----
                    dn = epi.tile([P, H1], dt.float32, tag="dn")
                    for ch in range(CH):
                        nc.tensor.matmul(out=dn[:, :], lhsT=s01_g[:, ch * P:(ch + 1) * P],
                                         rhs=exall[:, ch * H1:(ch + 1) * H1],
                                         start=(ch == 0), stop=(ch == CH - 1))
                    dninv = wk.tile([P, H1], dt.float32, tag="dninv")
                    nc.vector.reciprocal(out=dninv[:, :], in_=dn[:, :])
                    xagg = wk.tile([P, 8 * P], dt.bfloat16, tag="xagg")
                    for h in range(H1):
                        nc.vector.tensor_scalar(
                            out=xagg[:, h * P:(h + 1) * P], in0=acc[:, h * P:(h + 1) * P],
                            scalar1=dninv[:, h:h + 1], scalar2=None, op0=ALU.mult)
                    xaT = epi.tile([P, 8 * P], dt.bfloat16, tag="xaT")
                    for h in range(H1):
                        nc.tensor.transpose(out=xaT[:, h * P:(h + 1) * P],
                                            in_=xagg[:, h * P:(h + 1) * P],
                                            identity=id_sb[:, :])
                    xaTs = wk.tile([P, 8 * P], dt.bfloat16, tag="xaTs")
                    nc.vector.tensor_copy(out=xaTs[:, :], in_=xaT[:, :])
                    # hT blocks: hT[c128, d125] per 128-col block j of the 2048
                    hTs = wk.tile([P, 16 * P], dt.bfloat16, tag="hTs")
                    for half in range(2):
                        hT = epi.tile([P, 8 * P], dt.float32, tag="hT")
                        for jj in range(8):
                            j = half * 8 + jj
                            h = j // 2
                            nc.tensor.matmul(
                                out=hT[:, jj * P:jj * P + GN],
                                lhsT=wl1_sb[:, j * P:(j + 1) * P],
                                rhs=xaTs[:, h * P:h * P + GN],
                                start=True, stop=True)
                        # relu(h) drain, split across engines
                        if half == 0:
                            nc.scalar.activation(
                                out=hTs[:, :8 * P], in_=hT[:, :], func=AF.Relu)
                        else:
                            nc.vector.tensor_scalar(
                                out=hTs[:, 8 * P:], in0=hT[:, :], scalar1=1.0,
                                scalar2=0.0, op0=ALU.mult, op1=ALU.max)
                    # layer-2 projections: xl2 | xr2 = h @ Wl2 | h @ Wr2
                    l2 = accp.tile([P, 8 * P], dt.float32, tag="acc")
                    for j in range(16):
                        nc.tensor.matmul(out=l2[:GN, 0:D2],
                                         lhsT=hTs[:, j * P:j * P + GN],
                                         rhs=wl2_sb[:, j * D2:(j + 1) * D2],
                                         start=(j == 0), stop=(j == 15))
                    for j in range(16):
                        nc.tensor.matmul(out=l2[:GN, 512:512 + D2],
                                         lhsT=hTs[:, j * P:j * P + GN],
                                         rhs=wr2_sb[:, j * D2:(j + 1) * D2],
                                         start=(j == 0), stop=(j == 15))
                    xlr2sb = wk.tile([P, 2 * D2], dt.float32, tag="xlr2")
                    nc.vector.tensor_copy(out=xlr2sb[:GN, 0:D2], in_=l2[:GN, 0:D2])
                    nc.vector.tensor_copy(out=xlr2sb[:GN, D2:2 * D2],
                                          in_=l2[:GN, 512:512 + D2])
                    nc.vector.tensor_copy(out=xr2sb[:GN, g * D2:(g + 1) * D2],
                                          in_=l2[:GN, 512:512 + D2])
                    nc.sync.dma_start(out=xlr2[g * GN:(g + 1) * GN, :],
                                      in_=xlr2sb[:GN, :])

                # ============ exchange layer-2 src projections ============
                if "C" not in phases:
                    for g in range(G):
                        oz = wk.tile([P, D2], dt.float32, tag="o2")
                        nc.vector.memset(oz[:, :], 0)
                        nc.sync.dma_start(out=out[g * GN:(g + 1) * GN, :],
                                          in_=oz[:GN, :])
                if "A" in phases:
                  nc.gpsimd.collective_compute(
                    "AllGather", ALU.bypass, replica_groups=[list(range(M))],
                      ins=[xlr2[:, :]], outs=[ag[:, :]])
                  nc.sync.dma_start(out=xl2_tab[:, :], in_=ag[:, 0:D2])

                # ============ layer 2: edge pass ============
                for g in range(G if "C" in phases else 0):
                    g0 = g * CH * P
                    eaT_g = gin.tile([P, CH * P], dt.bfloat16, tag="eaT")
                    nc.sync.dma_start(out=eaT_g[:, :], in_=eaT[:, g0:g0 + CH * P])
                    s01_g = gin.tile([P, CH * P], dt.bfloat16, tag="s01")
                    nc.sync.dma_start(out=s01_g[:, :], in_=s01N[:, g0:g0 + CH * P])
                    s1T_g = gin.tile([P, CH * P], dt.bfloat16, tag="s1T")
                    nc.sync.dma_start(out=s1T_g[:, :], in_=s01T[:, g0:g0 + CH * P])
                    si_g = wk.tile([P, CH], dt.int32, tag="si")
                    nc.sync.dma_start(out=si_g[:, :], in_=srci[:, g * CH:(g + 1) * CH])

                    acc2 = accp.tile([P, 8 * P], dt.float32, tag="acc")
                    for ch in range(CH):
                        sl = slice(ch * P, (ch + 1) * P)
                        xl2e = wk3.tile([P, D2], dt.float32, tag="xl2e")
                        nc.gpsimd.indirect_dma_start(
                            out=xl2e[:, :], out_offset=None, in_=xl2_tab[:, :],
                            in_offset=bass.IndirectOffsetOnAxis(
                                ap=si_g[:, ch:ch + 1], axis=0))
                        q2 = qp.tile([P, 512], dt.float32, tag="q")
                        nc.tensor.matmul(out=q2[:, 0:D2], lhsT=eaT_g[:, sl],
                                         rhs=we2_sb[:, :], start=True, stop=False)
                        nc.tensor.matmul(out=q2[:, 0:D2], lhsT=s1T_g[:GN, sl],
                                         rhs=xr2sb[:GN, g * D2:(g + 1) * D2],
                                         start=False, stop=True)
                        q2b = wk3.tile([P, D2], dt.bfloat16, tag="q2b")
                        nc.vector.tensor_tensor(out=q2b[:, :], in0=xl2e[:, :],
                                                in1=q2[:, 0:D2], op=ALU.add)
                        m2 = wk3.tile([P, D2], dt.bfloat16, tag="m2")
                        nc.scalar.activation(out=m2[:, :], in_=q2b[:, :],
                                             func=AF.Prelu, alpha=NEG)
                        w2 = wk3.tile([P, D2], dt.bfloat16, tag="w2")
                        nc.vector.tensor_tensor(out=w2[:, :], in0=m2[:, :],
                                                in1=att2_sb[:, :], op=ALU.mult)
                        lg2 = wk3.tile([P, 1], dt.float32, tag="lg2")
                        nc.vector.tensor_reduce(out=lg2[:, :], in_=w2[:, :],
                                                axis=AX.X, op=ALU.add)
                        ex2 = wk3.tile([P, 1], dt.float32, tag="ex2")
                        nc.scalar.activation(out=ex2[:, :], in_=lg2[:, :], func=AF.Exp)
                        xls2 = wk3.tile([P, D2 + 1], dt.bfloat16, tag="xls2")
                        nc.vector.tensor_scalar(out=xls2[:, 0:D2], in0=xl2e[:, :],
                                                scalar1=ex2[:, 0:1], scalar2=None,
                                                op0=ALU.mult)
                        nc.vector.tensor_copy(out=xls2[:, D2:D2 + 1], in_=ex2[:, :])
                        nc.tensor.matmul(out=acc2[:, 0:D2 + 1], lhsT=s01_g[:, sl],
                                         rhs=xls2[:, :],
                                         start=(ch == 0), stop=(ch == CH - 1))
                    d2i = wk.tile([P, 1], dt.float32, tag="d2i")
                    nc.vector.reciprocal(out=d2i[:, :], in_=acc2[:, D2:D2 + 1])
                    o2 = wk.tile([P, D2], dt.float32, tag="o2")
                    nc.vector.tensor_scalar(out=o2[:, :], in0=acc2[:, 0:D2],
                                            scalar1=d2i[:, 0:1], scalar2=0.0,
                                            op0=ALU.mult, op1=ALU.max)
                    nc.sync.dma_start(out=out[g * GN:(g + 1) * GN, :], in_=o2[:GN, :])

    nc.compile()
    return nc


def _prep_inputs(x, edge_index, edge_attr, Wl1, bl1, Wr1, br1, We1, att1, b1,
                 Wl2, bl2, Wr2, br2, We2, att2, b2):
    for b in (bl1, br1, b1, bl2, br2, b2):
        assert not np.any(np.asarray(b)), "nonzero biases not implemented"

    x = np.asarray(x, np.float32)
    src = np.asarray(edge_index[0], dtype=np.int64)
    dst = np.asarray(edge_index[1], dtype=np.int64)
    ea = np.asarray(edge_attr, dtype=np.float32)
    order = np.argsort(dst, kind="stable")
    s_src, s_dst, s_ea = src[order], dst[order], ea[order]

    # self-loop attrs: mean of incoming edge attrs per dst (0 if none)
    nbounds = np.searchsorted(s_dst, np.arange(N + 1))
    ncnt = np.diff(nbounds)
    sums = np.zeros((N, F), np.float32)
    nz = ncnt > 0
    red = np.add.reduceat(s_ea, nbounds[:-1].clip(0, max(len(s_dst) - 1, 0)), axis=0)
    sums[nz] = red[nz]
    self_attr = sums / np.maximum(ncnt, 1)[:, None].astype(np.float32)

    # group boundaries: 80 groups of GN dst nodes; CH incl GN self edges
    gbounds = np.searchsorted(s_dst, np.arange(0, N + GN, GN))
    gcnts = np.diff(gbounds) + GN
    CH = int(np.max((gcnts + P - 1) // P))
    L = G * CH * P

    common = {
        "wl1": np.asarray(Wl1, np.float32).astype(BF16),
        "wr1": np.asarray(Wr1, np.float32).astype(BF16),
        "we1": np.asarray(We1, np.float32).astype(BF16),
        "att1s": np.tile(np.asarray(att1, np.float32).reshape(1, D1),
                         (P, 1)).astype(BF16),
        "wl2b": np.asarray(Wl2, np.float32).reshape(16, P, D2)
                .transpose(1, 0, 2).reshape(P, 16 * D2).astype(BF16),
        "wr2b": np.asarray(Wr2, np.float32).reshape(16, P, D2)
                .transpose(1, 0, 2).reshape(P, 16 * D2).astype(BF16),
        "we2": np.asarray(We2, np.float32).astype(BF16),
        "att2s": np.tile(np.asarray(att2, np.float32).reshape(1, D2),
                         (P, 1)).astype(BF16),
        "ident": np.eye(P, dtype=np.float32).astype(BF16),
    }

    in_maps = []
    for k in range(M):
        base = k * NPC
        esrc = np.zeros(L, np.int64)
        edst = np.zeros(L, np.int64)    # global dst
        eloc = np.full(L, -1, np.int64)  # dst-local in group (0..124), -1 pad
        eea = np.zeros((L, F), np.float32)
        for g in range(G):
            gi = k * G + g
            gb = base + g * GN
            lo, hi = gbounds[gi], gbounds[gi + 1]
            n_real = hi - lo
            n_tot = n_real + GN
            assert n_tot <= CH * P
            o0 = g * CH * P
            esrc[o0:o0 + n_real] = s_src[lo:hi]
            edst[o0:o0 + n_real] = s_dst[lo:hi]
            eloc[o0:o0 + n_real] = s_dst[lo:hi] - gb
            eea[o0:o0 + n_real] = s_ea[lo:hi]
            nn = np.arange(GN)
            esrc[o0 + n_real:o0 + n_tot] = gb + nn
            edst[o0 + n_real:o0 + n_tot] = gb + nn
            eloc[o0 + n_real:o0 + n_tot] = nn
            eea[o0 + n_real:o0 + n_tot] = self_attr[gb + nn]
        valid = eloc >= 0
        xsrc = np.where(valid[:, None], x[esrc], 0.0).astype(np.float32)
        xdst = np.where(valid[:, None], x[edst], 0.0).astype(np.float32)
        s01 = np.zeros((L, P), np.float32)
        vi = np.nonzero(valid)[0]
        s01[vi, eloc[vi]] = 1.0

        def blocked(rows):  # [L, F'] -> [P, L] chunk-blocked (e on partitions)
            Fw = rows.shape[1]
            return np.ascontiguousarray(
                rows.reshape(G * CH, P, Fw).transpose(1, 0, 2).reshape(P, G * CH * Fw))

        im = dict(common)
        im["xsrcT"] = np.ascontiguousarray(xsrc.T).astype(BF16)
        im["xdstT"] = np.ascontiguousarray(xdst.T).astype(BF16)
        im["eaT"] = np.ascontiguousarray(eea.T).astype(BF16)
        im["xsrcN"] = blocked(xsrc).astype(BF16)
        im["s01N"] = blocked(s01).astype(BF16)
        s01Tc = np.zeros((P, L), np.float32)
        s01Tc[eloc[vi], vi] = 1.0
        im["s01T"] = s01Tc.astype(BF16)
        im["srci"] = np.ascontiguousarray(
            esrc.reshape(G * CH, P).T).astype(np.int32)
        in_maps.append(im)
    return in_maps, CH


_PROG_CACHE = {}


def _get_program(CH, phases="ABC"):
    key = (CH, phases)
    if key not in _PROG_CACHE:
        _PROG_CACHE[key] = _build_program(CH, phases)
    return _PROG_CACHE[key]


def run(inputs, trace=False, tmpdir=None, phases="ABC"):
    in_maps, CH = _prep_inputs(**inputs)
    nc = _get_program(CH, phases)
    res = run_bass_kernel_spmd(nc, in_maps, list(range(M)), trace=trace,
                               tmpdir=tmpdir)
    outp = np.concatenate([res.results[k]["out"] for k in range(M)], axis=0)
    return outp.astype(np.float32), res


def kernel(**inputs):
    outp, _ = run(inputs)
    return outp
